# revision 13
# baseline (speedup 1.0000x reference)
"""Trainium2 Bass kernel for nn_CrossAttentionLayer (B=4, N=1024, M=4096,
DQ=DC=1024, H=16, DH=64).

Sharding: 8 cores = 4 batches x 2 half-head-groups. Core c handles batch
c//2 and heads [8*(c%2), 8*(c%2)+8). Each core computes its partial
out = concat_heads(attn) @ Wo_slice; host sums the two partials per batch
and adds the bias.

All attention operands are bf16 (validated max-rel-err 3.5e-3 vs the f32
reference, tolerance 2e-2): x^T/ctx^T/Wq/Wk/Wv are cast + transposed on
the host, so the kernel does NO PE transposes and K^T/Q^T/V stay resident
in SBUF (no DRAM spill).  Math identity on device:
  P = clamp(exp(scale*S + madd), e^-5, e^5)   with madd = 0 / -1000 (mask)
equals exp(clip(where(mask, scale*S, -inf), -5, 5)).  The softmax
denominator comes from a 65th ones-column appended to V (row 64 of each
[65, n] P@V output accumulates sum(P)).

Engine plan per core: PE does only the GEMMs (Q/K/V proj, S, P@V, out
proj); ScalarE does the exp (33.5M elems, the co-bottleneck) writing
bf16; DVE does the clamp (bf16 in-place) and all PSUM evacuations; S
uses 2x row tiling (tile_position (0,0)/(64,0)) so both heads of a pair
share the PE array.

Schedule: attention pass (pair0, n-half0) is interleaved into the K/V
projection loop so exp starts early; P@V for chunk mc is emitted after
chunk mc+1's S matmuls (1-step software pipeline) so the in-order PE
queue never waits on the exp+clamp latency; each pass's normalization is
deferred until after the next pass's first S; ctx^T is prefetched on the
SP DMA queue while x^T/madd use the Activation queue and weights the
GPSIMD queue.
"""
import sys
sys.path.insert(0, '/opt/trn_rl_repo')
from contextlib import ExitStack

import numpy as np
import ml_dtypes

import concourse.bass as bass  # noqa: F401
import concourse.mybir as mybir
import concourse.tile as tile
from concourse import bacc
from concourse.bass_utils import run_bass_kernel_spmd

F32 = mybir.dt.float32
F32R = mybir.dt.float32r
BF16 = mybir.dt.bfloat16
AF = mybir.ActivationFunctionType
ALU = mybir.AluOpType

B, N, M = 4, 1024, 4096
DQ = 1024
NHC = 8              # heads per core
D = 64
IC = NHC * D         # 512 inner dims per core
NP = NHC // 2        # 4 head pairs per core
MC = M // 128        # 32 context chunks of 128
E5 = float(np.exp(np.float32(5.0)))
EM5 = float(np.exp(np.float32(-5.0)))
SCALE = float(D) ** -0.5  # 0.125

_CACHE = {}


def _emit(nc, tc, tensors, pfx=""):
    xt_d, ctxt_d, wq_d, wk_d, wv_d, wo_d, madd_d, out_d = tensors

    with nc.allow_low_precision(reason="bf16 matmul operands"), ExitStack() as ctx:
        persist = ctx.enter_context(tc.tile_pool(name=f"{pfx}persist", bufs=1))

        madd_sb = persist.tile([128, MC], F32, tag="madd")
        nc.scalar.dma_start(madd_sb[:], madd_d[:])
        ones_f = persist.tile([128, 1], F32, tag="onesf")
        nc.vector.memset(ones_f[:], 1.0)
        ones_b = persist.tile([128, 1], BF16, tag="onesb")
        nc.vector.tensor_copy(ones_b[:], ones_f[:])
        ones_r = persist.tile([1, 64], F32R, tag="onesr")
        nc.vector.tensor_copy(ones_r[:], ones_f[0:1, 0:1].to_broadcast((1, 64)))

        QT = [persist.tile([128, N], BF16, tag=f"qt{p}", name=f"{pfx}qt{p}")
              for p in range(NP)]
        KT = [persist.tile([128, M], BF16, tag=f"kt{p}", name=f"{pfx}kt{p}")
              for p in range(NP)]
        V = [persist.tile([128, NHC * 65], BF16, tag=f"v{mc}", name=f"{pfx}v{mc}")
             for mc in range(MC)]
        OnT = [persist.tile([128, N], F32R, tag=f"ont{p}", name=f"{pfx}ont{p}")
               for p in range(NP)]

        # all inputs live in the persistent pool and their DMAs issue up
        # front: nothing shares SBUF with them, so no load waits on compute.
        # x^T first (gates the Q projection), then wq/wk/ctx0 (gate K0),
        # then the rest.
        xt_s = persist.tile([128, 8, N], BF16, tag="xt")
        nc.scalar.dma_start(xt_s[:], xt_d.rearrange("(c p) n -> p c n", p=128))
        wq_r = persist.tile([128, 8, IC], BF16, tag="wq")
        nc.gpsimd.dma_start(wq_r[:], wq_d.rearrange("(c p) i -> p c i", p=128))
        wk_r = persist.tile([128, 8, IC], BF16, tag="wk")
        nc.gpsimd.dma_start(wk_r[:], wk_d.rearrange("(c p) i -> p c i", p=128))
        wv_r = persist.tile([128, 8, IC], BF16, tag="wv")
        nc.gpsimd.dma_start(wv_r[:], wv_d.rearrange("(c p) i -> p c i", p=128))
        wo_r = persist.tile([128, NP, DQ], F32R, tag="wo")

        # ones columns of V never change: write them once up front
        for mc in range(MC):
            v3 = V[mc].rearrange("q (h e) -> q h e", e=65)
            nc.vector.tensor_copy(
                v3[:, :, 64:65], ones_b[:, 0:1, None].to_broadcast((128, NHC, 1)))

        # ctx^T staging pool opened early so the first blocks prefetch
        # during the Q projection
        pcl = ctx.enter_context(tc.tile_pool(name=f"{pfx}pcl", bufs=3))

        def stage_ctx(m5):
            t = pcl.tile([128, 8, 512], BF16, tag="ctxs", name=f"{pfx}cs{m5}")
            nc.sync.dma_start(
                t[:], ctxt_d[:, m5 * 512:(m5 + 1) * 512].rearrange(
                    "(c p) m -> p c m", p=128))
            return t

        def norm_head(O2, p, nh, prb, psR, rtags):
            """Normalize one pass's [65, 512] O tiles into OnT[p]."""
            for h2 in range(2):
                rc = prb.tile([1, 512], F32R, tag="rc",
                              name=f"{pfx}rc{p}{nh}{h2}")
                nc.vector.reciprocal(rc[:], O2[h2][64:65, :])
                Rb = psR.tile([64, 512], F32, tag=rtags[h2],
                              name=f"{pfx}rb{p}{nh}{h2}")
                nc.tensor.matmul(Rb[:], ones_r[:], rc[:], start=True, stop=True)
                rbs = prb.tile([64, 512], F32, tag="rbs",
                               name=f"{pfx}rbs{p}{nh}{h2}")
                nc.vector.tensor_copy(rbs[:], Rb[:])
                nc.vector.tensor_tensor(
                    OnT[p][h2 * 64:(h2 + 1) * 64, nh * 512:(nh + 1) * 512],
                    O2[h2][0:64, :], rbs[:], ALU.mult)

        def att_S(p, nh, mc, psS, pp):
            """S matmuls -> exp -> clamp; returns the bf16 P tile."""
            S = psS.tile([128, 1024], F32, tag="s", name=f"{pfx}s{p}_{nh}_{mc}")
            nc.tensor.matmul(S[:, 0:512], KT[p][0:64, mc * 128:(mc + 1) * 128],
                             QT[p][0:64, nh * 512:(nh + 1) * 512],
                             start=True, stop=True, tile_position=(0, 0))
            nc.tensor.matmul(S[:, 512:1024],
                             KT[p][64:128, mc * 128:(mc + 1) * 128],
                             QT[p][64:128, nh * 512:(nh + 1) * 512],
                             start=True, stop=True, tile_position=(64, 0))
            P_sb = pp.tile([128, 1024], BF16, tag="p",
                           name=f"{pfx}p{p}_{nh}_{mc}")
            nc.scalar.activation(P_sb[:], S[:], AF.Exp,
                                 bias=madd_sb[:, mc:mc + 1], scale=SCALE)
            nc.vector.tensor_scalar(P_sb[:], P_sb[:], E5, EM5, ALU.min, ALU.max)
            return P_sb

        def att_PV(p, mc, P_sb, O):
            for h2 in range(2):
                h = 2 * p + h2
                nc.tensor.matmul(O[h2][:], V[mc][:, h * 65:(h + 1) * 65],
                                 P_sb[:, h2 * 512:(h2 + 1) * 512],
                                 start=(mc == 0), stop=(mc == MC - 1))

        # ---- Phase A: Q^T = (Wq^T x^T) from host-transposed x^T ----
        with tc.tile_pool(name=f"{pfx}psA", bufs=4, space="PSUM") as psA:
            ctx_tiles = {m5: stage_ctx(m5) for m5 in range(2)}
            for nh in range(2):
                for p in range(NP):
                    qp = psA.tile([128, 512], F32, tag="qp")
                    for dc in range(8):
                        nc.tensor.matmul(
                            qp[:], wq_r[:, dc, p * 128:(p + 1) * 128],
                            xt_s[:, dc, nh * 512:(nh + 1) * 512],
                            start=(dc == 0), stop=(dc == 7))
                    if (nh * NP + p) % 2 == 0:
                        nc.scalar.copy(QT[p][:, nh * 512:(nh + 1) * 512], qp[:])
                    else:
                        nc.vector.tensor_copy(
                            QT[p][:, nh * 512:(nh + 1) * 512], qp[:])

        # ---- Phase B: K^T/V projections + attention pass (pair0, nh0) ----
        with tc.tile_pool(name=f"{pfx}pp0", bufs=3) as pp0, \
             tc.tile_pool(name=f"{pfx}prb0", bufs=2) as prb0, \
             tc.tile_pool(name=f"{pfx}psKV", bufs=2, space="PSUM") as psKV, \
             tc.tile_pool(name=f"{pfx}psS0", bufs=2, space="PSUM") as psS0, \
             tc.tile_pool(name=f"{pfx}psO0", bufs=1, space="PSUM") as psO0:
            nc.gpsimd.dma_start(wo_r[:], wo_d.rearrange("(p q) d -> q p d", q=128))
            O0 = [psO0.tile([65, 512], F32, tag=f"o0_{h2}", name=f"{pfx}o0_{h2}")
                  for h2 in range(2)]
            pend = None      # (mc, P_sb) with P@V not yet emitted
            for m5 in range(8):
                if m5 + 2 < 8:
                    ctx_tiles[m5 + 2] = stage_ctx(m5 + 2)
                ctx_s = ctx_tiles.pop(m5)
                kp = psKV.tile([128, 512], F32, tag="kv", name=f"{pfx}k0_{m5}")
                for dc in range(8):
                    nc.tensor.matmul(kp[:], wk_r[:, dc, 0:128], ctx_s[:, dc, :],
                                     start=(dc == 0), stop=(dc == 7))
                nc.vector.tensor_copy(KT[0][:, m5 * 512:(m5 + 1) * 512], kp[:])
                for s in range(4):
                    mc = m5 * 4 + s
                    vp = psKV.tile([128, 512], F32, tag="kv",
                                   name=f"{pfx}v{m5}_{s}")
                    for dc in range(8):
                        nc.tensor.matmul(
                            vp[:], ctx_s[:, dc, s * 128:(s + 1) * 128],
                            wv_r[:, dc, :], start=(dc == 0), stop=(dc == 7))
                    v3 = V[mc].rearrange("q (h e) -> q h e", e=65)
                    nc.vector.tensor_copy(
                        v3[:, :, 0:64], vp[:].rearrange("q (h e) -> q h e", e=64))
                    P_sb = att_S(0, 0, mc, psS0, pp0)
                    if pend is not None:
                        att_PV(0, pend[0], pend[1], O0)
                    pend = (mc, P_sb)
            att_PV(0, pend[0], pend[1], O0)
            norm_head(O0, 0, 0, prb0, psS0, ("s", "s"))

        # ---- Phase C: remaining 7 attention passes (flat 1-step pipeline).
        # K1-3 projections ride inside the first three (ACT-bound) passes
        # with ctx^T re-staged; the nh0 half of the output projection rides
        # inside passes 5-6 once all nh0 norms have landed. ----
        with tc.tile_pool(name=f"{pfx}pf", bufs=4) as pf, \
             tc.tile_pool(name=f"{pfx}pp", bufs=3) as pp, \
             tc.tile_pool(name=f"{pfx}prb", bufs=2) as prb, \
             tc.tile_pool(name=f"{pfx}psS", bufs=2, space="PSUM") as psS, \
             tc.tile_pool(name=f"{pfx}psO", bufs=1, space="PSUM") as psO:
            state = {"pv": None, "norm": None}

            def stage_ctx2(mb, tagix):
                t = pcl.tile([128, 8, 512], BF16, tag="ctxs",
                             name=f"{pfx}cs{tagix}_{mb}")
                nc.sync.dma_start(
                    t[:], ctxt_d[:, mb * 512:(mb + 1) * 512].rearrange(
                        "(c p) m -> p c m", p=128))
                return t

            def out_group(n8, dqh, psF):
                po = psF.tile([128, 512], F32, tag="po",
                              name=f"{pfx}po{n8}_{dqh}")
                for p2 in range(NP):
                    nc.tensor.matmul(
                        po[:], OnT[p2][:, n8 * 128:(n8 + 1) * 128],
                        wo_r[:, p2, dqh * 512:(dqh + 1) * 512],
                        start=(p2 == 0), stop=(p2 == NP - 1))
                ob = pf.tile([128, 512], F32, tag="ob",
                             name=f"{pfx}ob{n8}_{dqh}")
                if (n8 * 2 + dqh) % 2 == 0:
                    nc.scalar.copy(ob[:], po[:])
                else:
                    nc.vector.tensor_copy(ob[:], po[:])
                nc.sync.dma_start(
                    out_d[n8 * 128:(n8 + 1) * 128,
                          dqh * 512:(dqh + 1) * 512], ob[:])

            def run_pass(p, nh, kpair=None, psKV2=None, kctx=None,
                         po_groups=None, psF=None):
                O_cur = [psO.tile([65, 512], F32, tag=f"oo{h2}",
                                  name=f"{pfx}o{p}_{nh}_{h2}")
                         for h2 in range(2)]
                for mc in range(MC):
                    P_sb = att_S(p, nh, mc, psS, pp)
                    if kpair is not None and mc % 4 == 0:
                        mb = mc // 4
                        if mb + 2 < 8:
                            kctx[mb + 2] = stage_ctx2(mb + 2, kpair)
                        cs = kctx.pop(mb)
                        kp = psKV2.tile([128, 512], F32, tag="kv",
                                        name=f"{pfx}ck{kpair}_{mb}")
                        for dc in range(8):
                            nc.tensor.matmul(
                                kp[:],
                                wk_r[:, dc, kpair * 128:(kpair + 1) * 128],
                                cs[:, dc, :], start=(dc == 0), stop=(dc == 7))
                        nc.vector.tensor_copy(
                            KT[kpair][:, mb * 512:(mb + 1) * 512], kp[:])
                    if po_groups and mc % 8 == 4:
                        out_group(*po_groups.pop(0), psF)
                    if state["pv"] is not None:
                        att_PV(*state["pv"])
                    state["pv"] = (p, mc, P_sb, O_cur)
                    if state["norm"] is not None and mc == 1:
                        norm_head(state["norm"][0], state["norm"][1],
                                  state["norm"][2], prb, psS, ("s", "s"))
                        state["norm"] = None
                state["norm"] = (O_cur, p, nh)

            with tc.tile_pool(name=f"{pfx}psKV2", bufs=2, space="PSUM") as psKV2:
                for i, (p, nh) in enumerate([(0, 1), (1, 0), (2, 0)]):
                    kpair = i + 1
                    kctx = {mb: stage_ctx2(mb, kpair) for mb in range(2)}
                    run_pass(p, nh, kpair=kpair, psKV2=psKV2, kctx=kctx)
            run_pass(3, 0)
            with tc.tile_pool(name=f"{pfx}psF", bufs=2, space="PSUM") as psF:
                po_groups = [(n8, dqh) for n8 in range(4) for dqh in range(2)]
                run_pass(1, 1, po_groups=po_groups, psF=psF)
                run_pass(2, 1, po_groups=po_groups, psF=psF)
                run_pass(3, 1)
                att_PV(*state["pv"])
                state["pv"] = None
                norm_head(state["norm"][0], state["norm"][1],
                          state["norm"][2], prb, psS, ("s", "s"))
                state["norm"] = None
                for n8 in range(4, 8):
                    for dqh in range(2):
                        out_group(n8, dqh, psF)


def _build(n_bodies=1):
    nc = bacc.Bacc("TRN2", target_bir_lowering=False, debug=False, num_devices=8)
    xt_d = nc.dram_tensor("xt", [DQ, N], BF16, kind="ExternalInput")
    ctxt_d = nc.dram_tensor("ctxt", [DQ, M], BF16, kind="ExternalInput")
    wq_d = nc.dram_tensor("wq", [DQ, IC], BF16, kind="ExternalInput")
    wk_d = nc.dram_tensor("wk", [DQ, IC], BF16, kind="ExternalInput")
    wv_d = nc.dram_tensor("wv", [DQ, IC], BF16, kind="ExternalInput")
    wo_d = nc.dram_tensor("wo", [IC, DQ], F32, kind="ExternalInput")
    madd_d = nc.dram_tensor("madd", [128, MC], F32, kind="ExternalInput")
    out_d = nc.dram_tensor("out", [N, DQ], F32, kind="ExternalOutput")
    with tile.TileContext(nc) as tc:
        for i in range(n_bodies):
            _emit(nc, tc, (xt_d, ctxt_d, wq_d, wk_d, wv_d, wo_d, madd_d, out_d),
                  pfx=(f"b{i}_" if n_bodies > 1 else ""))
    nc.compile()
    return nc


def _in_maps(x, context, mask, Wq, Wkv, Wo):
    bf = ml_dtypes.bfloat16
    maps = []
    for c in range(8):
        b, hh = divmod(c, 2)
        cs = hh * IC
        madd = np.where(mask[b], np.float32(0.0), np.float32(-1000.0))
        madd = madd.astype(np.float32).reshape(MC, 128).T
        maps.append({
            "xt": np.ascontiguousarray(x[b].T.astype(bf)),
            "ctxt": np.ascontiguousarray(context[b].T.astype(bf)),
            "wq": np.ascontiguousarray(Wq[:, cs:cs + IC].astype(bf)),
            "wk": np.ascontiguousarray(Wkv[:, cs:cs + IC].astype(bf)),
            "wv": np.ascontiguousarray(Wkv[:, DQ + cs:DQ + cs + IC].astype(bf)),
            "wo": np.ascontiguousarray(Wo[cs:cs + IC, :]),
            "madd": np.ascontiguousarray(madd),
        })
    return maps


def kernel(x, context, mask, Wq, Wkv, Wo, bo):
    x = np.asarray(x, dtype=np.float32)
    context = np.asarray(context, dtype=np.float32)
    mask = np.asarray(mask)
    Wq = np.asarray(Wq, dtype=np.float32)
    Wkv = np.asarray(Wkv, dtype=np.float32)
    Wo = np.asarray(Wo, dtype=np.float32)
    bo = np.asarray(bo, dtype=np.float32)

    if "nc" not in _CACHE:
        _CACHE["nc"] = _build()
    nc = _CACHE["nc"]

    res = run_bass_kernel_spmd(nc, _in_maps(x, context, mask, Wq, Wkv, Wo),
                               core_ids=list(range(8)))
    _CACHE["last_results"] = res

    out = np.empty((B, N, DQ), dtype=np.float32)
    for b in range(B):
        out[b] = res.results[2 * b]["out"] + res.results[2 * b + 1]["out"] \
            + bo[None, :]
    return out


# revision 17
# speedup vs baseline: 1.0245x; 1.0245x over previous
"""Trainium2 Bass kernel for nn_CrossAttentionLayer (B=4, N=1024, M=4096,
DQ=DC=1024, H=16, DH=64).

Sharding: 8 cores = 4 batches x 2 half-head-groups. Core c handles batch
c//2 and heads [8*(c%2), 8*(c%2)+8). Each core computes its partial
out = concat_heads(attn) @ Wo_slice; host sums the two partials per batch
and adds the bias.

All attention operands are bf16 (validated max-rel-err 3.5e-3 vs the f32
reference, tolerance 2e-2): x^T/ctx^T/Wq/Wk/Wv are cast + transposed on
the host, so the kernel does NO PE transposes and K^T/Q^T/V stay resident
in SBUF (no DRAM spill).  Math identity on device:
  P = clamp(exp(scale*S + madd), e^-5, e^5)   with madd = 0 / -1000 (mask)
equals exp(clip(where(mask, scale*S, -inf), -5, 5)).  The softmax
denominator comes from a 65th ones-column appended to V (row 64 of each
[65, n] P@V output accumulates sum(P)).

Engine plan per core: PE does only the GEMMs (Q/K/V proj, S, P@V, out
proj); ScalarE does the exp (33.5M elems, the co-bottleneck) writing
bf16; DVE does the clamp (bf16 in-place) and all PSUM evacuations; S
uses 2x row tiling (tile_position (0,0)/(64,0)) so both heads of a pair
share the PE array.

Schedule: attention pass (pair0, n-half0) is interleaved into the K/V
projection loop so exp starts early; P@V for chunk mc is emitted after
chunk mc+1's S matmuls (1-step software pipeline) so the in-order PE
queue never waits on the exp+clamp latency; each pass's normalization is
deferred until after the next pass's first S; ctx^T is prefetched on the
SP DMA queue while x^T/madd use the Activation queue and weights the
GPSIMD queue.
"""
import sys
sys.path.insert(0, '/opt/trn_rl_repo')
from contextlib import ExitStack

import numpy as np
import ml_dtypes

import concourse.bass as bass  # noqa: F401
import concourse.mybir as mybir
import concourse.tile as tile
from concourse import bacc
from concourse.bass_utils import run_bass_kernel_spmd

F32 = mybir.dt.float32
F32R = mybir.dt.float32r
BF16 = mybir.dt.bfloat16
AF = mybir.ActivationFunctionType
ALU = mybir.AluOpType

B, N, M = 4, 1024, 4096
DQ = 1024
NHC = 8              # heads per core
D = 64
IC = NHC * D         # 512 inner dims per core
NP = NHC // 2        # 4 head pairs per core
MC = M // 128        # 32 context chunks of 128
E5 = float(np.exp(np.float32(5.0)))
EM5 = float(np.exp(np.float32(-5.0)))
SCALE = float(D) ** -0.5  # 0.125

_CACHE = {}


def _emit(nc, tc, tensors, pfx=""):
    xt_d, ctxt_d, wq_d, wk_d, wv_d, wo_d, madd_d, out_d = tensors

    with nc.allow_low_precision(reason="bf16 matmul operands"), ExitStack() as ctx:
        persist = ctx.enter_context(tc.tile_pool(name=f"{pfx}persist", bufs=1))

        madd_sb = persist.tile([128, MC], F32, tag="madd")
        nc.scalar.dma_start(madd_sb[:], madd_d[:])
        ones_f = persist.tile([128, 1], F32, tag="onesf")
        nc.vector.memset(ones_f[:], 1.0)
        ones_b = persist.tile([128, 1], BF16, tag="onesb")
        nc.vector.tensor_copy(ones_b[:], ones_f[:])
        ones_r = persist.tile([1, 64], F32R, tag="onesr")
        nc.vector.tensor_copy(ones_r[:], ones_f[0:1, 0:1].to_broadcast((1, 64)))

        QT = [persist.tile([128, N], BF16, tag=f"qt{p}", name=f"{pfx}qt{p}")
              for p in range(NP)]
        KT = [persist.tile([128, M], BF16, tag=f"kt{p}", name=f"{pfx}kt{p}")
              for p in range(NP)]
        V = [persist.tile([128, NHC * 65], BF16, tag=f"v{mc}", name=f"{pfx}v{mc}")
             for mc in range(MC)]
        OnT = [persist.tile([128, N], F32R, tag=f"ont{p}", name=f"{pfx}ont{p}")
               for p in range(NP)]

        # all inputs live in the persistent pool and their DMAs issue up
        # front: nothing shares SBUF with them, so no load waits on compute.
        # ctx0/wk/wv first (gate the early K0/V block below), then x^T/wq
        # (gate the Q projection), then the rest.
        wk_r = persist.tile([128, 8, IC], BF16, tag="wk")
        nc.gpsimd.dma_start(wk_r[:], wk_d.rearrange("(c p) i -> p c i", p=128))
        wv_r = persist.tile([128, 8, IC], BF16, tag="wv")
        nc.gpsimd.dma_start(wv_r[:], wv_d.rearrange("(c p) i -> p c i", p=128))
        xt_s = persist.tile([128, 8, N], BF16, tag="xt")
        nc.scalar.dma_start(
            xt_s[:, :, 0:512],
            xt_d[:, 0:512].rearrange("(c p) n -> p c n", p=128))
        wq_r = persist.tile([128, 8, IC], BF16, tag="wq")
        nc.gpsimd.dma_start(wq_r[:], wq_d.rearrange("(c p) i -> p c i", p=128))
        nc.scalar.dma_start(
            xt_s[:, :, 512:1024],
            xt_d[:, 512:1024].rearrange("(c p) n -> p c n", p=128))
        wo_r = persist.tile([128, NP, DQ], F32R, tag="wo")

        # ones columns of V never change: write them once up front
        for mc in range(MC):
            v3 = V[mc].rearrange("q (h e) -> q h e", e=65)
            nc.vector.tensor_copy(
                v3[:, :, 64:65], ones_b[:, 0:1, None].to_broadcast((128, NHC, 1)))

        # ctx^T staging pool opened early so the first blocks prefetch
        # during the Q projection
        pcl = ctx.enter_context(tc.tile_pool(name=f"{pfx}pcl", bufs=3))

        def stage_ctx(m5):
            t = pcl.tile([128, 8, 512], BF16, tag="ctxs", name=f"{pfx}cs{m5}")
            nc.sync.dma_start(
                t[:], ctxt_d[:, m5 * 512:(m5 + 1) * 512].rearrange(
                    "(c p) m -> p c m", p=128))
            return t

        def norm_head(O2, p, nh, prb, psR, rtags):
            """Normalize one pass's [65, 512] O tiles into OnT[p]."""
            for h2 in range(2):
                rc = prb.tile([1, 512], F32R, tag="rc",
                              name=f"{pfx}rc{p}{nh}{h2}")
                nc.vector.reciprocal(rc[:], O2[h2][64:65, :])
                Rb = psR.tile([64, 512], F32, tag=rtags[h2],
                              name=f"{pfx}rb{p}{nh}{h2}")
                nc.tensor.matmul(Rb[:], ones_r[:], rc[:], start=True, stop=True)
                rbs = prb.tile([64, 512], F32, tag="rbs",
                               name=f"{pfx}rbs{p}{nh}{h2}")
                nc.vector.tensor_copy(rbs[:], Rb[:])
                nc.vector.tensor_tensor(
                    OnT[p][h2 * 64:(h2 + 1) * 64, nh * 512:(nh + 1) * 512],
                    O2[h2][0:64, :], rbs[:], ALU.mult)

        def att_S(p, nh, mc, psS, pp):
            """S matmuls -> exp -> clamp; returns the bf16 P tile."""
            S = psS.tile([128, 1024], F32, tag="s", name=f"{pfx}s{p}_{nh}_{mc}")
            nc.tensor.matmul(S[:, 0:512], KT[p][0:64, mc * 128:(mc + 1) * 128],
                             QT[p][0:64, nh * 512:(nh + 1) * 512],
                             start=True, stop=True, tile_position=(0, 0))
            nc.tensor.matmul(S[:, 512:1024],
                             KT[p][64:128, mc * 128:(mc + 1) * 128],
                             QT[p][64:128, nh * 512:(nh + 1) * 512],
                             start=True, stop=True, tile_position=(64, 0))
            P_sb = pp.tile([128, 1024], BF16, tag="p",
                           name=f"{pfx}p{p}_{nh}_{mc}")
            nc.scalar.activation(P_sb[:], S[:], AF.Exp,
                                 bias=madd_sb[:, mc:mc + 1], scale=SCALE)
            nc.vector.tensor_scalar(P_sb[:], P_sb[:], E5, EM5, ALU.min, ALU.max)
            return P_sb

        def att_PV(p, mc, P_sb, O):
            for h2 in range(2):
                h = 2 * p + h2
                nc.tensor.matmul(O[h2][:], V[mc][:, h * 65:(h + 1) * 65],
                                 P_sb[:, h2 * 512:(h2 + 1) * 512],
                                 start=(mc == 0), stop=(mc == MC - 1))

        # ---- Phase A0: K0/V projections for context block 0 run before the
        # Q projection — PE starts as soon as ctx0+wk+wv land (~5us) ----
        kvstack = ctx.enter_context(ExitStack())
        psKV = kvstack.enter_context(
            tc.tile_pool(name=f"{pfx}psKV", bufs=2, space="PSUM"))

        def k0_block(m5, ctx_s):
            kp = psKV.tile([128, 512], F32, tag="kv", name=f"{pfx}k0_{m5}")
            for dc in range(8):
                nc.tensor.matmul(kp[:], wk_r[:, dc, 0:128], ctx_s[:, dc, :],
                                 start=(dc == 0), stop=(dc == 7))
            nc.vector.tensor_copy(KT[0][:, m5 * 512:(m5 + 1) * 512], kp[:])

        def v_chunk(m5, s, ctx_s):
            mc = m5 * 4 + s
            vp = psKV.tile([128, 512], F32, tag="kv", name=f"{pfx}v{m5}_{s}")
            for dc in range(8):
                nc.tensor.matmul(vp[:], ctx_s[:, dc, s * 128:(s + 1) * 128],
                                 wv_r[:, dc, :], start=(dc == 0), stop=(dc == 7))
            v3 = V[mc].rearrange("q (h e) -> q h e", e=65)
            nc.vector.tensor_copy(v3[:, :, 0:64],
                                  vp[:].rearrange("q (h e) -> q h e", e=64))

        ctx0 = stage_ctx(0)
        k0_block(0, ctx0)
        for s in range(4):
            v_chunk(0, s, ctx0)
        ctx_tiles = {1: stage_ctx(1)}

        # ---- Phase A: Q^T = (Wq^T x^T) from host-transposed x^T ----
        with tc.tile_pool(name=f"{pfx}psA", bufs=4, space="PSUM") as psA:
            for nh in range(2):
                for p in range(NP):
                    qp = psA.tile([128, 512], F32, tag="qp")
                    for dc in range(8):
                        nc.tensor.matmul(
                            qp[:], wq_r[:, dc, p * 128:(p + 1) * 128],
                            xt_s[:, dc, nh * 512:(nh + 1) * 512],
                            start=(dc == 0), stop=(dc == 7))
                    if (nh * NP + p) % 2 == 0:
                        nc.scalar.copy(QT[p][:, nh * 512:(nh + 1) * 512], qp[:])
                    else:
                        nc.vector.tensor_copy(
                            QT[p][:, nh * 512:(nh + 1) * 512], qp[:])

        # ---- Phase B: K^T/V projections + attention pass (pair0, nh0) ----
        with tc.tile_pool(name=f"{pfx}pp0", bufs=3) as pp0, \
             tc.tile_pool(name=f"{pfx}prb0", bufs=2) as prb0, \
             tc.tile_pool(name=f"{pfx}psS0", bufs=2, space="PSUM") as psS0, \
             tc.tile_pool(name=f"{pfx}psO0", bufs=1, space="PSUM") as psO0:
            nc.gpsimd.dma_start(wo_r[:], wo_d.rearrange("(p q) d -> q p d", q=128))
            O0 = [psO0.tile([65, 512], F32, tag=f"o0_{h2}", name=f"{pfx}o0_{h2}")
                  for h2 in range(2)]
            pend = None      # (mc, P_sb) with P@V not yet emitted
            for m5 in range(8):
                if m5 + 2 < 8:
                    ctx_tiles[m5 + 2] = stage_ctx(m5 + 2)
                ctx_s = None if m5 == 0 else ctx_tiles.pop(m5)
                if ctx_s is not None:
                    k0_block(m5, ctx_s)
                for s in range(4):
                    mc = m5 * 4 + s
                    if ctx_s is not None:
                        v_chunk(m5, s, ctx_s)
                    P_sb = att_S(0, 0, mc, psS0, pp0)
                    if pend is not None:
                        att_PV(0, pend[0], pend[1], O0)
                    pend = (mc, P_sb)
            att_PV(0, pend[0], pend[1], O0)
            norm_head(O0, 0, 0, prb0, psS0, ("s", "s"))
        kvstack.close()

        # ---- Phase C: remaining 7 attention passes (flat 1-step pipeline).
        # K1-3 projections ride inside the first three (ACT-bound) passes
        # with ctx^T re-staged; the nh0 half of the output projection rides
        # inside passes 5-6 once all nh0 norms have landed. ----
        with tc.tile_pool(name=f"{pfx}pf", bufs=4) as pf, \
             tc.tile_pool(name=f"{pfx}pp", bufs=3) as pp, \
             tc.tile_pool(name=f"{pfx}prb", bufs=2) as prb, \
             tc.tile_pool(name=f"{pfx}psS", bufs=2, space="PSUM") as psS, \
             tc.tile_pool(name=f"{pfx}psO", bufs=1, space="PSUM") as psO:
            state = {"pv": None, "norm": None}

            def stage_ctx2(mb, tagix):
                t = pcl.tile([128, 8, 512], BF16, tag="ctxs",
                             name=f"{pfx}cs{tagix}_{mb}")
                nc.sync.dma_start(
                    t[:], ctxt_d[:, mb * 512:(mb + 1) * 512].rearrange(
                        "(c p) m -> p c m", p=128))
                return t

            def out_group(n8, dqh, psF):
                po = psF.tile([128, 512], F32, tag="po",
                              name=f"{pfx}po{n8}_{dqh}")
                for p2 in range(NP):
                    nc.tensor.matmul(
                        po[:], OnT[p2][:, n8 * 128:(n8 + 1) * 128],
                        wo_r[:, p2, dqh * 512:(dqh + 1) * 512],
                        start=(p2 == 0), stop=(p2 == NP - 1))
                ob = pf.tile([128, 512], F32, tag="ob",
                             name=f"{pfx}ob{n8}_{dqh}")
                if (n8 * 2 + dqh) % 2 == 0:
                    nc.scalar.copy(ob[:], po[:])
                else:
                    nc.vector.tensor_copy(ob[:], po[:])
                nc.sync.dma_start(
                    out_d[n8 * 128:(n8 + 1) * 128,
                          dqh * 512:(dqh + 1) * 512], ob[:])

            def run_pass(p, nh, kpair=None, psKV2=None, kctx=None,
                         po_groups=None, psF=None):
                O_cur = [psO.tile([65, 512], F32, tag=f"oo{h2}",
                                  name=f"{pfx}o{p}_{nh}_{h2}")
                         for h2 in range(2)]
                for mc in range(MC):
                    P_sb = att_S(p, nh, mc, psS, pp)
                    if kpair is not None and mc % 4 == 0:
                        mb = mc // 4
                        if mb + 2 < 8:
                            kctx[mb + 2] = stage_ctx2(mb + 2, kpair)
                        cs = kctx.pop(mb)
                        kp = psKV2.tile([128, 512], F32, tag="kv",
                                        name=f"{pfx}ck{kpair}_{mb}")
                        for dc in range(8):
                            nc.tensor.matmul(
                                kp[:],
                                wk_r[:, dc, kpair * 128:(kpair + 1) * 128],
                                cs[:, dc, :], start=(dc == 0), stop=(dc == 7))
                        nc.vector.tensor_copy(
                            KT[kpair][:, mb * 512:(mb + 1) * 512], kp[:])
                    if po_groups and mc % 8 == 4:
                        out_group(*po_groups.pop(0), psF)
                    if state["pv"] is not None:
                        att_PV(*state["pv"])
                    state["pv"] = (p, mc, P_sb, O_cur)
                    if state["norm"] is not None and mc == 1:
                        norm_head(state["norm"][0], state["norm"][1],
                                  state["norm"][2], prb, psS, ("s", "s"))
                        state["norm"] = None
                state["norm"] = (O_cur, p, nh)

            with tc.tile_pool(name=f"{pfx}psKV2", bufs=2, space="PSUM") as psKV2:
                for i, (p, nh) in enumerate([(0, 1), (1, 0), (2, 0)]):
                    kpair = i + 1
                    kctx = {mb: stage_ctx2(mb, kpair) for mb in range(2)}
                    run_pass(p, nh, kpair=kpair, psKV2=psKV2, kctx=kctx)
            run_pass(3, 0)
            with tc.tile_pool(name=f"{pfx}psF", bufs=2, space="PSUM") as psF:
                po_groups = [(n8, dqh) for n8 in range(4) for dqh in range(2)]
                run_pass(1, 1, po_groups=po_groups, psF=psF)
                run_pass(2, 1, po_groups=po_groups, psF=psF)
                run_pass(3, 1)
                att_PV(*state["pv"])
                state["pv"] = None
                norm_head(state["norm"][0], state["norm"][1],
                          state["norm"][2], prb, psS, ("s", "s"))
                state["norm"] = None
                for n8 in range(4, 8):
                    for dqh in range(2):
                        out_group(n8, dqh, psF)


def _build(n_bodies=1):
    nc = bacc.Bacc("TRN2", target_bir_lowering=False, debug=False, num_devices=8)
    xt_d = nc.dram_tensor("xt", [DQ, N], BF16, kind="ExternalInput")
    ctxt_d = nc.dram_tensor("ctxt", [DQ, M], BF16, kind="ExternalInput")
    wq_d = nc.dram_tensor("wq", [DQ, IC], BF16, kind="ExternalInput")
    wk_d = nc.dram_tensor("wk", [DQ, IC], BF16, kind="ExternalInput")
    wv_d = nc.dram_tensor("wv", [DQ, IC], BF16, kind="ExternalInput")
    wo_d = nc.dram_tensor("wo", [IC, DQ], F32, kind="ExternalInput")
    madd_d = nc.dram_tensor("madd", [128, MC], F32, kind="ExternalInput")
    out_d = nc.dram_tensor("out", [N, DQ], F32, kind="ExternalOutput")
    with tile.TileContext(nc) as tc:
        for i in range(n_bodies):
            _emit(nc, tc, (xt_d, ctxt_d, wq_d, wk_d, wv_d, wo_d, madd_d, out_d),
                  pfx=(f"b{i}_" if n_bodies > 1 else ""))
    nc.compile()
    return nc


def _in_maps(x, context, mask, Wq, Wkv, Wo):
    bf = ml_dtypes.bfloat16
    maps = []
    for c in range(8):
        b, hh = divmod(c, 2)
        cs = hh * IC
        madd = np.where(mask[b], np.float32(0.0), np.float32(-1000.0))
        madd = madd.astype(np.float32).reshape(MC, 128).T
        maps.append({
            "xt": np.ascontiguousarray(x[b].T.astype(bf)),
            "ctxt": np.ascontiguousarray(context[b].T.astype(bf)),
            "wq": np.ascontiguousarray(Wq[:, cs:cs + IC].astype(bf)),
            "wk": np.ascontiguousarray(Wkv[:, cs:cs + IC].astype(bf)),
            "wv": np.ascontiguousarray(Wkv[:, DQ + cs:DQ + cs + IC].astype(bf)),
            "wo": np.ascontiguousarray(Wo[cs:cs + IC, :]),
            "madd": np.ascontiguousarray(madd),
        })
    return maps


def kernel(x, context, mask, Wq, Wkv, Wo, bo):
    x = np.asarray(x, dtype=np.float32)
    context = np.asarray(context, dtype=np.float32)
    mask = np.asarray(mask)
    Wq = np.asarray(Wq, dtype=np.float32)
    Wkv = np.asarray(Wkv, dtype=np.float32)
    Wo = np.asarray(Wo, dtype=np.float32)
    bo = np.asarray(bo, dtype=np.float32)

    if "nc" not in _CACHE:
        _CACHE["nc"] = _build()
    nc = _CACHE["nc"]

    res = run_bass_kernel_spmd(nc, _in_maps(x, context, mask, Wq, Wkv, Wo),
                               core_ids=list(range(8)))
    _CACHE["last_results"] = res

    out = np.empty((B, N, DQ), dtype=np.float32)
    for b in range(B):
        out[b] = res.results[2 * b]["out"] + res.results[2 * b + 1]["out"] \
            + bo[None, :]
    return out
